# revision 12
# baseline (speedup 1.0000x reference)
"""Trainium2 Bass kernel for nn_LogicReasoningEncoder (GNN message passing).

Sharding: 8 cores = 4 batches x 2 target-node halves. Each core owns the
edges whose target node falls in its half, bucketed into 8 fixed-capacity
node blocks of 128 nodes so that every core runs the identical SPMD program.
Scatter-softmax is reformulated without the max pass (att is leaky-relu
bounded, so exp never overflows) and the alpha normalization is deferred to
a per-node divide after aggregation, so cross-core work is just one
pair-wise AllGather of updated node features per layer.

v3 structure notes:
- Layer 0 is algebraic: h_init[src] = 1_D*[src==0], so its gather is gone.
  Host groups src==0 edges at the front of each node block ("covered"
  chunks); the W0*(h_src.hr), (W1+W2)-colsum terms reduce to ncov small
  matmuls per block, and layers 1-2 get W2-colsum the same way.
- The scatter matmul's moving operand is [exab | w*rmsgT], so out col 0
  accumulates the softmax denominator (sm) and cols 1:129 the weighted
  message sum - no separate sm matmuls or sm PSUM tile.
- The edge mask is folded into the attention bias (attrelp += (em-1)*1e9),
  removing the per-slab em multiply; rsqrt in the LN tail is exp(-ln/2) so
  the whole steady state stays on one activation table set.
"""

import os
import sys
import numpy as np

for _p in ("/opt/trn_rl_repo", "/root/.axon_site/_ro/trn_rl_repo"):
    if _p not in sys.path:
        sys.path.append(_p)

import concourse.bass as bass
import concourse.mybir as mybir
from concourse import bacc, tile
from concourse.bass_utils import run_bass_kernel_spmd

# NOTE: SWDGE queue 1 corrupts gathers on HW (verified empirically); use queue 0.
GQ = int(os.environ.get("KQUEUE", "0"))
SP = bool(int(os.environ.get("KSP", "0")))

F32 = mybir.dt.float32
BF16 = mybir.dt.bfloat16
I16 = mybir.dt.int16
BF16_NP = mybir.dt.np(BF16)

B, N, E, D, L, NREL, TAU = 4, 2048, 32768, 128, 3, 1000, 0.1
NHALF = N // 2          # nodes per core
NBLK = NHALF // 128     # 8 node blocks per core
P = 128
NRELP = 1024            # rel table padded to 8 tokens x 128 ranks


# ----------------------------------------------------------------------------
# Host-side sharding / layout prep (index manipulation + layout only)
# ----------------------------------------------------------------------------

def _chunkify(x, ecap):
    """[ecap] -> [128, ecap//128] with x[c*128+p] at [p, c]."""
    return np.ascontiguousarray(x.reshape(ecap // 128, 128).T)


def _wrap16(x, ecap):
    """[ecap] -> int16 [128, ecap//16]: x[i] at [i%16, i//16], tiled x8 for Q7 cores."""
    w = np.ascontiguousarray(x.astype(np.int16).reshape(ecap // 16, 16).T)
    return np.ascontiguousarray(np.tile(w, (8, 1)))


def prepare_core_inputs(inputs):
    ei = np.asarray(inputs["edge_index"])          # [B, 2, E] int32
    rels = np.asarray(inputs["rels"])              # [B, E]
    scores = np.asarray(inputs["scores"])          # [B, E] f32
    cm = np.asarray(inputs["edge_conf_mask"])      # [B, E] bool
    em = np.asarray(inputs["edge_mask"])           # [B, E] bool
    conf = np.asarray(inputs["conf_embeds"])       # [B, E, D] f32

    # Fixed per-block edge capacity, uniform across all cores (SPMD).
    max_cnt = 0
    max_src0 = 1
    per_core = []
    for core in range(8):
        b, half = core // 2, core % 2
        base = half * NHALF
        tgt = ei[b, 1]
        sel = np.nonzero((tgt >= base) & (tgt < base + NHALF))[0]
        ltgt = tgt[sel] - base
        blk = ltgt >> 7
        cnts = np.bincount(blk, minlength=NBLK)
        max_cnt = max(max_cnt, int(cnts.max()))
        src0 = ei[b, 0][sel] == 0
        if src0.any():
            c0 = np.bincount(blk[src0], minlength=NBLK)
            max_src0 = max(max_src0, int(c0.max()))
        per_core.append((b, base, sel, ltgt, blk))

    e_blk = ((max_cnt + 383) // 384) * 384
    ecap = NBLK * e_blk
    ncov = (max_src0 + 127) // 128          # covered chunks per block

    rel_pad = np.zeros((NRELP, D), np.float32)
    rel_pad[:NREL] = np.asarray(inputs["rel_table"])
    rel_bf = rel_pad.astype(BF16_NP)

    in_maps = []
    for core in range(8):
        b, base, sel, ltgt, blk = per_core[core]
        src_g = ei[b, 0][sel]
        # block-sorted, src==0 edges first within each block
        key = blk.astype(np.int64) * 2 + (src_g != 0)
        order = np.argsort(key, kind="stable")
        perm = sel[order]                 # original edge ids
        lt = ltgt[order]
        bk = blk[order]
        cnts = np.bincount(bk, minlength=NBLK)
        slot = np.concatenate(
            [bb * e_blk + np.arange(cnts[bb]) for bb in range(NBLK)]
        ).astype(np.int64) if len(perm) else np.zeros(0, np.int64)

        src_p = np.zeros(ecap, np.int32)
        rels_p = np.zeros(ecap, np.int32)
        scores_p = np.zeros(ecap, np.float32)
        em_p = np.zeros(ecap, np.float32)
        cm_p = np.zeros(ecap, np.float32)
        conf_p = np.zeros((ecap, D), np.float32)
        oh = np.zeros((ecap, P), np.float32)

        src_p[slot] = ei[b, 0][perm]
        rels_p[slot] = rels[b][perm]
        scores_p[slot] = scores[b][perm]
        em_p[slot] = em[b][perm].astype(np.float32)
        cm_p[slot] = cm[b][perm].astype(np.float32)
        conf_p[slot] = conf[b][perm]
        j = lt - (slot // e_blk) * 128
        oh[slot, j] = 1.0
        # partition-major one-hot: [p, c*128 + j]
        oh_pm = np.ascontiguousarray(
            oh.reshape(ecap // 128, 128, 128).transpose(1, 0, 2).reshape(P, ecap)
        ).astype(BF16_NP)

        hr_fm = np.ascontiguousarray(rel_bf[rels_p].T)      # [D, ecap]
        srcz = (src_p == 0)
        covcols = (np.arange(NBLK)[:, None] * e_blk
                   + np.arange(ncov * 128)[None, :]).reshape(-1)
        hrz_cov = np.ascontiguousarray(
            (hr_fm[:, covcols].astype(np.float32)
             * srcz[covcols][None, :]).astype(BF16_NP))       # [D, NBLK*ncov*128]
        srcz_cov = srcz[covcols].astype(BF16_NP).reshape(1, -1)

        hown = np.zeros((P, NBLK * 128), np.float32)
        if core % 2 == 0:
            hown[0, 0:128] = 1.0  # node 0 lives at block 0, partition 0

        m = {
            "conf_fm": np.ascontiguousarray(conf_p.T).astype(BF16_NP),
            "hr_fm": hr_fm,
            "onehot_pm": oh_pm,
            "hrz_cov": hrz_cov,
            "srcz_cov": srcz_cov,
            "scores_ch": _chunkify(scores_p, ecap),
            "em_ch": _chunkify(em_p, ecap),
            "cm_ch": _chunkify(cm_p, ecap),
            "src_wr": _wrap16(src_p, ecap),
            "hown0": hown,
            "ident_b": np.eye(P).astype(BF16_NP),
            "rq": np.asarray(inputs["r_query_embed"])[b].reshape(D, 1).astype(BF16_NP),
            "msg_W": np.asarray(inputs["msg_W"]).astype(BF16_NP),
            "msg_b_col": np.ascontiguousarray(np.asarray(inputs["msg_b"]).T).astype(np.float32),
            "upd_W": np.asarray(inputs["upd_W"]).astype(BF16_NP),
            "upd_b_row": np.asarray(inputs["upd_b"]).reshape(L, D).astype(BF16_NP),
            "ln_g_row": np.asarray(inputs["ln_g"]).reshape(1, D).astype(np.float32),
            "ln_b_row": np.asarray(inputs["ln_b"]).reshape(1, D).astype(np.float32),
            "attbeta4": np.ascontiguousarray(np.concatenate(
                [np.asarray(inputs["att_W"])[:, P:2 * P, 0].T,     # a2_k [D,3]
                 np.asarray(inputs["beta_W"]).reshape(D, 1)], axis=1)).astype(BF16_NP),
            "a1": np.ascontiguousarray(
                np.asarray(inputs["att_W"])[:, 0:P, 0].T).astype(BF16_NP),   # [D,3]
            "aq6": np.ascontiguousarray(np.concatenate(
                [np.asarray(inputs["att_W"])[:, 2 * P:3 * P, 0].T,  # a3_k [D,3]
                 np.asarray(inputs["beta_W"]).reshape(D, 1),
                 np.zeros((D, 2), np.float32)], axis=1)).astype(BF16_NP),
            "sc_bias": np.concatenate([
                np.asarray(inputs["att_b"]).reshape(-1),       # 3
                np.asarray(inputs["beta_b"]).reshape(-1),      # 1
                np.asarray(inputs["den_b2"]).reshape(-1),      # 1
                np.zeros(1, np.float32),
            ]).reshape(1, 6).astype(BF16_NP),
            "den_W1": np.asarray(inputs["den_W1"]).astype(BF16_NP),
            "den_b1_row": np.asarray(inputs["den_b1"]).reshape(1, D).astype(BF16_NP),
            "den_W2": np.asarray(inputs["den_W2"]).astype(BF16_NP),
        }
        in_maps.append(m)
    return in_maps, ecap, ncov


# ----------------------------------------------------------------------------
# Device program
# ----------------------------------------------------------------------------

def build_program(ecap, ncov=1, reps=1, ablate=()):
    ab = set(ablate)
    C = ecap // 128          # chunks
    SLAB = 3072
    NSLAB = ecap // SLAB     # slabs (6 tiles each)
    ST = SLAB // 512         # tiles per slab (6)
    SC = SLAB // 128         # chunks per slab (24)
    e_blk = ecap // NBLK
    CPB = e_blk // 128       # chunks per node block

    # covered chunk -> (global tile index, col offset) for the src==0 terms
    cov_sites = []           # list of (t, o, bb, i)
    for bb in range(NBLK):
        for i in range(ncov):
            e0 = bb * e_blk + i * 128
            cov_sites.append((e0 // 512, e0 % 512, bb, i))
    cov_by_tile = {}
    for t, o, bb, i in cov_sites:
        cov_by_tile.setdefault(t, []).append((o, bb, i))

    nc = bacc.Bacc("TRN2", num_devices=8, debug=False, num_swdge_queues=2)

    dp = nc.declare_dram_parameter
    conf_fm_d = dp("conf_fm", [D, ecap], BF16, isOutput=False)
    hr_fm_d = dp("hr_fm", [D, ecap], BF16, isOutput=False)
    onehot_pm = dp("onehot_pm", [P, ecap], BF16, isOutput=False)
    hrz_cov_d = dp("hrz_cov", [D, NBLK * ncov * 128], BF16, isOutput=False)
    srcz_cov_d = dp("srcz_cov", [1, NBLK * ncov * 128], BF16, isOutput=False)
    scores_ch_d = dp("scores_ch", [P, C], F32, isOutput=False)
    em_ch_d = dp("em_ch", [P, C], F32, isOutput=False)
    cm_ch_d = dp("cm_ch", [P, C], F32, isOutput=False)
    src_wr_d = dp("src_wr", [128, ecap // 16], I16, isOutput=False)
    hown0_d = dp("hown0", [P, NBLK * 128], F32, isOutput=False)
    ident_b_d = dp("ident_b", [P, P], BF16, isOutput=False)
    rq_d = dp("rq", [D, 1], BF16, isOutput=False)
    msg_W_d = dp("msg_W", [L, 5 * D, D], BF16, isOutput=False)
    msg_b_col_d = dp("msg_b_col", [D, L], F32, isOutput=False)
    upd_W_d = dp("upd_W", [L, D, D], BF16, isOutput=False)
    upd_b_row_d = dp("upd_b_row", [L, D], BF16, isOutput=False)
    ln_g_row_d = dp("ln_g_row", [1, D], F32, isOutput=False)
    ln_b_row_d = dp("ln_b_row", [1, D], F32, isOutput=False)
    attbeta4_d = dp("attbeta4", [D, 4], BF16, isOutput=False)
    a1_d = dp("a1", [D, L], BF16, isOutput=False)
    aq6_d = dp("aq6", [D, 6], BF16, isOutput=False)
    sc_bias_d = dp("sc_bias", [1, 6], BF16, isOutput=False)
    den_W1_d = dp("den_W1", [3 * D, D], BF16, isOutput=False)
    den_b1_row_d = dp("den_b1_row", [1, D], BF16, isOutput=False)
    den_W2_d = dp("den_W2", [D, 1], BF16, isOutput=False)
    out_d = dp("out", [L, D], F32, isOutput=True)

    # DRAM scratch
    hhalf = nc.dram_tensor("hhalf", [NHALF, D], BF16)
    hfull = [nc.dram_tensor(f"hfull{i}", [N, D], BF16) for i in range(2)]

    AF = mybir.ActivationFunctionType
    ALU = mybir.AluOpType

    with tile.TileContext(nc) as tc:
        for _rep in range(reps):
            with (
                tc.tile_pool(name=f"res{_rep}", bufs=1) as res,
                tc.tile_pool(name=f"wgt{_rep}", bufs=1) as wgt,
            ):
                # ---------------- persistent SBUF ----------------
                hr_fm = res.tile([P, ecap], BF16)
                conf_fm = res.tile([P, ecap], BF16)
                oh_fm = res.tile([P, ecap], BF16)
                s_ch = res.tile([P, C], F32)
                attrelp = res.tile([P, L, C], F32)
                src_wr = res.tile([128, ecap // 16], I16)
                hrz_cov = res.tile([P, NBLK * ncov * 128], BF16)
                srcz_cov = res.tile([1, NBLK * ncov * 128], BF16)
                h_tiles = [res.tile([P, NBLK, 128], F32, name=f"h_t{i}", tag=f"h_t{i}")
                           for i in range(2)]

                # ---------------- weights in SBUF ----------------
                msgW = wgt.tile([P, L, 5, D], BF16)
                denW = wgt.tile([P, 3, D], BF16)      # A, B, C blocks of den_W1
                updW = wgt.tile([P, L, D], BF16)
                a1 = wgt.tile([P, L], BF16)
                attbeta4 = wgt.tile([P, 4], BF16)     # a2_0..2, beta_W
                aq6 = wgt.tile([P, 6], BF16)          # a3_0..2, beta_W, 0, 0
                denW2 = wgt.tile([P, 1], BF16)
                msgb = wgt.tile([P, L], F32)
                wrow = wgt.tile([1, L, D], BF16)      # k=0: sum(W1+W2); k>0: sum(W2)
                updb_row = wgt.tile([1, L, D], BF16)
                denb1_row = wgt.tile([1, D], BF16)
                rq_bf = wgt.tile([P, 1], BF16)
                scb_bf = wgt.tile([1, 6], BF16)
                ident_b = wgt.tile([P, P], BF16)
                ones_col = wgt.tile([P, 1], BF16)
                ones_r1b = wgt.tile([1, P], BF16)
                ones_r1f = wgt.tile([1, P], F32)
                ones11 = wgt.tile([1, 1], BF16)
                eps_col = wgt.tile([P, 1], F32)
                g_rep = wgt.tile([P, P], F32)
                b_rep = wgt.tile([P, P], F32)
                rep6 = wgt.tile([P, 6], F32)
                den_bias = wgt.tile([P, 1], F32)
                row6_bf = wgt.tile([1, 6], BF16)

                gp, sy, ve, sc, te = nc.gpsimd, nc.sync, nc.vector, nc.scalar, nc.tensor

                # ---------------- step 0: load + cast weights ----------------
                sy.dma_start(msgW[:], msg_W_d[:].rearrange("k (t i) o -> i k t o", i=P))
                sy.dma_start(denW[:], den_W1_d[:].rearrange("(t i) o -> i t o", i=P))
                sy.dma_start(updW[:], upd_W_d[:].rearrange("k i o -> i k o"))
                sy.dma_start(a1[:], a1_d[:])
                sy.dma_start(attbeta4[:], attbeta4_d[:])
                sy.dma_start(aq6[:], aq6_d[:])
                sy.dma_start(denW2[:], den_W2_d[:])
                sy.dma_start(msgb[:], msg_b_col_d[:])
                sy.dma_start(updb_row[:], upd_b_row_d[:].rearrange("k d -> () k d"))
                sy.dma_start(denb1_row[:], den_b1_row_d[:])
                sy.dma_start(rq_bf[:], rq_d[:])
                sy.dma_start(scb_bf[:], sc_bias_d[:])
                sy.dma_start(ident_b[:], ident_b_d[:])
                sy.dma_start(hrz_cov[:], hrz_cov_d[:])
                sy.dma_start(srcz_cov[:], srcz_cov_d[:])
                ve.memset(ones_col[:], 1.0)
                ve.memset(ones_r1b[:], 1.0)
                ve.memset(ones_r1f[:], 1.0)
                ve.memset(ones11[:], 1.0)
                ve.memset(eps_col[:], 1e-5)
                sy.dma_start(src_wr[:], src_wr_d[:])
                sy.dma_start(h_tiles[0][:].rearrange("p b d -> p (b d)"), hown0_d[:])

                with tc.tile_pool(name=f"prep_ps{_rep}", bufs=1, space="PSUM") as pps:
                    # wrow_k: k=0 -> ones^T(W1_0+W2_0); k>0 -> ones^T W2_k
                    w3ps = pps.tile([1, L, D], F32)
                    te.matmul(w3ps[:, 0, :], ones_col[:], msgW[:, 0, 1, :],
                              start=True, stop=False)
                    te.matmul(w3ps[:, 0, :], ones_col[:], msgW[:, 0, 2, :],
                              start=False, stop=True)
                    for k in range(1, L):
                        te.matmul(w3ps[:, k, :], ones_col[:], msgW[:, k, 2, :])
                    sc.copy(wrow[:], w3ps[:])

                    # row6 = rq^T @ [a3_0,a3_1,a3_2,beta_W,0,0] + sc_bias
                    r6ps = pps.tile([1, 6], F32)
                    te.matmul(r6ps[:], rq_bf[:], aq6[:], start=True, stop=False)
                    te.matmul(r6ps[:], ones11[:], scb_bf[:], start=False, stop=True)
                    sc.copy(row6_bf[:], r6ps[:])

                    # rep6 = ones x row6 ; den_bias = denB^T rq + den_b1
                    rp6 = pps.tile([P, 6], F32)
                    te.matmul(rp6[:], ones_r1b[:], row6_bf[:])
                    ve.tensor_copy(rep6[:], rp6[:])

                    dbp = pps.tile([P, 1], F32)
                    te.matmul(dbp[:], denW[:, 1, :], rq_bf[:], start=True, stop=False)
                    te.matmul(dbp[:], denb1_row[:], ones11[:], start=False, stop=True)
                    ve.tensor_copy(den_bias[:], dbp[:])

                    # g_rep / b_rep (fp32 broadcast matmuls)
                    lng = wgt.tile([1, D], F32, name="lng_row")
                    lnb = wgt.tile([1, D], F32, name="lnb_row")
                    sy.dma_start(lng[:], ln_g_row_d[:])
                    sy.dma_start(lnb[:], ln_b_row_d[:])
                    grp = pps.tile([P, D], F32)
                    te.matmul(grp[:], ones_r1f[:], lng[:])
                    ve.tensor_copy(g_rep[:], grp[:])
                    brp = pps.tile([P, D], F32)
                    te.matmul(brp[:], ones_r1f[:], lnb[:])
                    ve.tensor_copy(b_rep[:], brp[:])

                # ---------------- phase A: den gate, att_rel ------
                with (
                    tc.tile_pool(name=f"pA3{_rep}", bufs=3) as pA3,
                    tc.tile_pool(name=f"pA_ps{_rep}", bufs=2, space="PSUM") as pAps,
                    tc.tile_pool(name=f"chA{_rep}", bufs=1) as chA,
                ):
                    scores_ch = chA.tile([P, C], F32)
                    cm_ch = chA.tile([P, C], F32)
                    em_ch = chA.tile([P, C], F32)
                    betarel_ch = chA.tile([P, C], F32)
                    denlin_ch = chA.tile([P, C], F32)
                    sy.dma_start(scores_ch[:], scores_ch_d[:])
                    sy.dma_start(cm_ch[:], cm_ch_d[:])
                    sy.dma_start(em_ch[:], em_ch_d[:])

                    for s in range(NSLAB):
                        lo = s * SLAB
                        sy.dma_start(hr_fm[:, lo:lo + SLAB], hr_fm_d[:, lo:lo + SLAB])
                        sy.dma_start(conf_fm[:, lo:lo + SLAB], conf_fm_d[:, lo:lo + SLAB])
                        sy.dma_start(oh_fm[:, lo:lo + SLAB], onehot_pm[:, lo:lo + SLAB])

                    for s in range(NSLAB):
                        r5_ps = pAps.tile([P, SC, 5], F32, tag="r5ps")
                        hids = []
                        for tt in range(ST):
                            t = s * ST + tt
                            e0 = t * 512
                            dps = pAps.tile([P, 512], F32, tag="denps")
                            te.matmul(dps[:], denW[:, 0, :], hr_fm[:, e0:e0 + 512],
                                      start=True, stop=False)
                            te.matmul(dps[:], denW[:, 2, :], conf_fm[:, e0:e0 + 512],
                                      start=False, stop=True)
                            hid = pA3.tile([P, 512], BF16, tag="hid")
                            sc.activation(hid[:], dps[:], AF.Relu, bias=den_bias[:])
                            hids.append(hid)
                            # att_rel / beta projections, chunk layout
                            for j in range(4):
                                cc = 4 * tt + j
                                te.matmul(r5_ps[:, cc, 0:4],
                                          hr_fm[:, e0 + 128 * j:e0 + 128 * j + 128],
                                          attbeta4[:])
                            # den hidden -> den_lin, one tile behind (hides relu)
                            if tt > 0:
                                for j in range(4):
                                    cc = 4 * (tt - 1) + j
                                    te.matmul(r5_ps[:, cc, 4:5],
                                              hids[tt - 1][:, 128 * j:128 * j + 128],
                                              denW2[:])
                        for j in range(4):
                            cc = 4 * (ST - 1) + j
                            te.matmul(r5_ps[:, cc, 4:5],
                                      hids[ST - 1][:, 128 * j:128 * j + 128],
                                      denW2[:])

                        c0 = s * SC
                        for k in range(L):
                            ve.tensor_copy(attrelp[:, k, c0:c0 + SC], r5_ps[:, :, k])
                        ve.tensor_copy(betarel_ch[:, c0:c0 + SC], r5_ps[:, :, 3])
                        ve.tensor_copy(denlin_ch[:, c0:c0 + SC], r5_ps[:, :, 4])

                    # chunk-layout gate math
                    beta_t = chA.tile([P, C], F32)
                    sc.activation(beta_t[:], betarel_ch[:], AF.Sigmoid, bias=rep6[:, 3:4])
                    tmp_t = chA.tile([P, C], F32)
                    ve.tensor_tensor(tmp_t[:], scores_ch[:], beta_t[:], ALU.subtract)
                    gk_t = chA.tile([P, C], F32)
                    sc.activation(gk_t[:], tmp_t[:], AF.Sigmoid, scale=1.0 / TAU)
                    ve.tensor_scalar(gk_t[:], gk_t[:], -0.5, None, ALU.add)
                    ve.tensor_tensor(gk_t[:], cm_ch[:], gk_t[:], ALU.mult)
                    ve.tensor_scalar(gk_t[:], gk_t[:], 0.5, None, ALU.add)   # gate
                    den_t = chA.tile([P, C], F32)
                    sc.activation(den_t[:], denlin_ch[:], AF.Sigmoid, bias=rep6[:, 4:5])
                    ve.tensor_tensor(s_ch[:], gk_t[:], den_t[:], ALU.mult)
                    ve.tensor_tensor(s_ch[:], s_ch[:], em_ch[:], ALU.mult)
                    # attrelp += rep6_k + (em-1)*1e9   (edge mask folded into bias)
                    maskadd = chA.tile([P, C], F32)
                    ve.tensor_scalar(maskadd[:], em_ch[:], 1.0e9, -1.0e9,
                                     ALU.mult, ALU.add)
                    for k in range(L):
                        ve.tensor_scalar(attrelp[:, k, :], attrelp[:, k, :],
                                         rep6[:, k:k + 1], None, ALU.add)
                        ve.tensor_tensor(attrelp[:, k, :], attrelp[:, k, :],
                                         maskadd[:], ALU.add)

                # ---------------- phase B: layers ----------------
                for k in range(L):
                    with tc.tile_pool(name=f"ups{k}_{_rep}", bufs=1, space="PSUM") as upool:
                      usum_ps = upool.tile([P, NBLK, 256], F32)
                      with (
                        tc.tile_pool(name=f"lps{k}_{_rep}", bufs=1, space="PSUM") as lpool,
                        tc.tile_pool(name=f"sl{k}_{_rep}", bufs=6) as slp,
                        tc.tile_pool(name=f"tp{k}_{_rep}", bufs=3) as tpp,
                        tc.tile_pool(name=f"rm{k}_{_rep}", bufs=8) as rmp,
                      ):
                        # prefetch all slab gathers for this layer up front:
                        # edge-major gather (contiguous 256B per descriptor)
                        # + one xbar DMA-transpose per slab to feature-major
                        hsrcs = []
                        if k > 0:
                            h_read = hfull[(k - 1) % 2]
                            for s in range(NSLAB):
                                lo = s * SLAB
                                hsrc_sl = slp.tile([P, SLAB], BF16, tag="hsrc",
                                                   bufs=4)
                                if "gather" in ab:
                                    gp.dma_start(hsrc_sl[:], onehot_pm[:, lo:lo + SLAB])
                                else:
                                    gp.dma_gather(
                                        hsrc_sl[:].rearrange("p (o e) -> p o e", o=1),
                                        h_read[:],
                                        src_wr[:, lo // 16:(lo + SLAB) // 16],
                                        SLAB, SLAB, D, transpose=True,
                                        single_packet=SP, queue_num=GQ,
                                    )
                                hsrcs.append(hsrc_sl)

                        for s in range(NSLAB):
                            att_ps = lpool.tile([P, SC], F32, tag="attps", bufs=1)
                            rms = []
                            for tt in range(ST):
                                t = s * ST + tt
                                e0, f0 = t * 512, tt * 512
                                mps = lpool.tile([P, 512], F32, tag="msgps", bufs=2)
                                if k > 0:
                                    prod = tpp.tile([P, 512], BF16, tag="prod")
                                    ve.tensor_tensor(prod[:],
                                                     hsrcs[s][:, f0:f0 + 512],
                                                     hr_fm[:, e0:e0 + 512], ALU.mult)
                                    te.matmul(mps[:], msgW[:, k, 0, :], prod[:],
                                              start=True, stop=False)
                                    te.matmul(mps[:], msgW[:, k, 1, :],
                                              hsrcs[s][:, f0:f0 + 512],
                                              start=False, stop=False)
                                    te.matmul(mps[:], msgW[:, k, 3, :],
                                              hr_fm[:, e0:e0 + 512],
                                              start=False, stop=False)
                                else:
                                    te.matmul(mps[:], msgW[:, k, 3, :],
                                              hr_fm[:, e0:e0 + 512],
                                              start=True, stop=False)
                                # src==0 corrections on covered chunks
                                for (o, bb, i) in cov_by_tile.get(t, ()):
                                    cs = (bb * ncov + i) * 128
                                    if k == 0:
                                        te.matmul(mps[:, o:o + 128], msgW[:, 0, 0, :],
                                                  hrz_cov[:, cs:cs + 128],
                                                  start=False, stop=False,
                                                  skip_group_check=True)
                                    te.matmul(mps[:, o:o + 128], wrow[:, k, :],
                                              srcz_cov[:, cs:cs + 128],
                                              start=False, stop=False,
                                              skip_group_check=True)
                                te.matmul(mps[:], msgW[:, k, 4, :],
                                          conf_fm[:, e0:e0 + 512],
                                          start=False, stop=True)

                                rmsg = rmp.tile([P, 512], BF16, tag="rmsg")
                                sc.activation(rmsg[:], mps[:], AF.Relu,
                                              bias=msgb[:, k:k + 1])
                                rms.append(rmsg)
                                # att cols, one tile behind (hides relu latency)
                                if tt > 0:
                                    for j in range(4):
                                        cc = 4 * (tt - 1) + j
                                        te.matmul(att_ps[:, cc:cc + 1],
                                                  rms[tt - 1][:, 128 * j:128 * j + 128],
                                                  a1[:, k:k + 1])
                            for j in range(4):
                                cc = 4 * (ST - 1) + j
                                te.matmul(att_ps[:, cc:cc + 1],
                                          rms[ST - 1][:, 128 * j:128 * j + 128],
                                          a1[:, k:k + 1])

                            # softmax chain for this slab, chunk layout
                            c0 = s * SC
                            att_sl = tpp.tile([P, SC], F32, tag="att_sl")
                            ve.tensor_tensor(att_sl[:], att_ps[:],
                                             attrelp[:, k, c0:c0 + SC], ALU.add)
                            lr_t = tpp.tile([P, SC], F32, tag="lrt")
                            ve.tensor_scalar(lr_t[:], att_sl[:], 0.01, None,
                                             ALU.mult)
                            ve.tensor_tensor(att_sl[:], att_sl[:], lr_t[:],
                                             ALU.max)
                            sc.activation(att_sl[:], att_sl[:], AF.Exp)
                            w_sl = tpp.tile([P, SC], F32, tag="w_sl")
                            ve.tensor_tensor(w_sl[:], att_sl[:],
                                             s_ch[:, c0:c0 + SC], ALU.mult)

                            # transpose + build moving operand [exab | w*rmsgT]
                            for tt in range(ST if "scatter" not in ab else 0):
                                t = s * ST + tt
                                trp = lpool.tile([P, 4, P], BF16, tag="trps",
                                                 bufs=1)
                                for j in range(4):
                                    te.transpose(trp[:, j, :],
                                                 rms[tt][:, 128 * j:128 * j + 128],
                                                 ident_b[:])
                                mv = tpp.tile([P, 4, 132], BF16, tag="mv")
                                ve.tensor_copy(
                                    mv[:, :, 0:1],
                                    att_sl[:, 4 * tt:4 * tt + 4].rearrange(
                                        "p c -> p c ()"))
                                ve.tensor_tensor(
                                    mv[:, :, 1:129], trp[:],
                                    w_sl[:, 4 * tt:4 * tt + 4].rearrange(
                                        "p c -> p c ()").broadcast_to([P, 4, P]),
                                    ALU.mult)
                                for j in range(4):
                                    cc = 4 * t + j
                                    bb = cc // CPB
                                    te.matmul(usum_ps[:, bb, 0:129],
                                              oh_fm[:, cc * 128:cc * 128 + 128],
                                              mv[:, j, 0:129],
                                              start=(cc % CPB == 0),
                                              stop=(cc % CPB == CPB - 1))

                      if "scatter" in ab:
                          for bb2 in range(NBLK):
                              te.matmul(usum_ps[:, bb2, 0:129], ident_b[:],
                                        oh_fm[:, 0:129], start=True, stop=True)
                      # ---------------- layer tail ----------------
                      with (
                          tc.tile_pool(name=f"tl{k}_{_rep}", bufs=1) as tlp,
                          tc.tile_pool(name=f"tlps{k}_{_rep}", bufs=1, space="PSUM") as tlps,
                      ):
                            sm_s = tlp.tile([P, NBLK], F32)
                            ve.tensor_scalar(sm_s[:], usum_ps[:, :, 0], 1e-8,
                                             None, ALU.add)
                            rsm = tlp.tile([P, NBLK], F32)
                            ve.reciprocal(rsm[:], sm_s[:])
                            aggr = tlp.tile([P, NBLK, P], BF16)
                            ve.tensor_tensor(aggr[:], usum_ps[:, :, 1:129],
                                             rsm[:].broadcast_to([P, NBLK, P]), ALU.mult)
                            aggrT = tlp.tile([P, NBLK, P], BF16)
                            trp2 = tlps.tile([P, NBLK, P], BF16, bufs=1, tag="tr2")
                            for bb in range(NBLK):
                                te.transpose(trp2[:, bb, :], aggr[:, bb, :], ident_b[:])
                            ve.tensor_copy(aggrT[:], trp2[:])

                            hb_ps = tlps.tile([P, NBLK, P], F32, tag="hb")
                            for bb in range(NBLK):
                                te.matmul(hb_ps[:, bb, :], aggrT[:, bb, :], updW[:, k, :],
                                          start=True, stop=False)
                                te.matmul(hb_ps[:, bb, :], ones_r1b[:], updb_row[:, k, :],
                                          start=False, stop=True)

                            hs = tlp.tile([P, NBLK, P], F32)
                            ve.tensor_tensor(hs[:], hb_ps[:], h_tiles[k % 2][:], ALU.add)
                            mu = tlp.tile([P, NBLK], F32)
                            ve.tensor_reduce(mu[:], hs[:], mybir.AxisListType.X, ALU.add)
                            ve.tensor_scalar(mu[:], mu[:], 1.0 / P, None, ALU.mult)
                            xc = tlp.tile([P, NBLK, P], F32)
                            ve.tensor_tensor(xc[:], hs[:], mu[:].broadcast_to([P, NBLK, P]),
                                             ALU.subtract)
                            sq = tlp.tile([P, NBLK, P], F32)
                            sc.activation(sq[:], xc[:], AF.Square)
                            var = tlp.tile([P, NBLK], F32)
                            ve.tensor_reduce(var[:], sq[:], mybir.AxisListType.X, ALU.add)
                            ve.tensor_scalar(var[:], var[:], 1.0 / P, None, ALU.mult)
                            # rsqrt via exp(-0.5*ln(x)): stays in the exp
                            # act-table set (Sqrt would force a table reload)
                            sd = tlp.tile([P, NBLK], F32)
                            sc.activation(sd[:], var[:], AF.Ln, bias=eps_col[:])
                            rsd = tlp.tile([P, NBLK], F32)
                            sc.activation(rsd[:], sd[:], AF.Exp, scale=-0.5)
                            hn = h_tiles[(k + 1) % 2]
                            ve.tensor_tensor(hn[:], xc[:], rsd[:].broadcast_to([P, NBLK, P]),
                                             ALU.mult)
                            ve.tensor_tensor(hn[:], hn[:],
                                             g_rep[:].rearrange("p d -> p () d").broadcast_to([P, NBLK, P]),
                                             ALU.mult)
                            ve.tensor_tensor(hn[:], hn[:],
                                             b_rep[:].rearrange("p d -> p () d").broadcast_to([P, NBLK, P]),
                                             ALU.add)

                            sy.dma_start(out_d[k:k + 1, :], hn[0:1, 0, :])

                            if k < L - 1:
                                hstage = tlp.tile([P, NBLK, P], BF16)
                                ve.tensor_copy(hstage[:], hn[:])
                                sy.dma_start(
                                    hhalf[:].rearrange("(b p) d -> p b d", p=P),
                                    hstage[:],
                                )
                                if "cc" in ab:
                                    gp.dma_start(hfull[k % 2][0:NHALF, :], hhalf[:])
                                elif "ccsmall" in ab:
                                    # timing probe: 1/8-size exchange (wrong results)
                                    gp.collective_compute(
                                        "AllGather",
                                        ALU.bypass,
                                        replica_groups=[[0, 1], [2, 3], [4, 5], [6, 7]],
                                        ins=[hhalf[0:NHALF // 8, :].opt()],
                                        outs=[hfull[k % 2][0:N // 8, :].opt()],
                                    )
                                else:
                                    gp.collective_compute(
                                        "AllGather",
                                        ALU.bypass,
                                        replica_groups=[[0, 1], [2, 3], [4, 5], [6, 7]],
                                        ins=[hhalf[:].opt()],
                                        outs=[hfull[k % 2][:].opt()],
                                    )


    nc.compile()
    return nc


_PROGRAM_CACHE = {}


def _get_program(ecap, ncov):
    key = (ecap, ncov)
    if key not in _PROGRAM_CACHE:
        _PROGRAM_CACHE[key] = build_program(ecap, ncov)
    return _PROGRAM_CACHE[key]


def kernel(**inputs):
    in_maps, ecap, ncov = prepare_core_inputs(inputs)
    nc = _get_program(ecap, ncov)
    res = run_bass_kernel_spmd(nc, in_maps, list(range(8)))
    outs = np.stack([np.asarray(res.results[2 * b]["out"]) for b in range(B)], axis=0)
    return outs.astype(np.float32)
